# revision 10
# baseline (speedup 1.0000x reference)
"""Causal multi-head attention (B=4, H=16, S=2048, D=128, fp32) on 8 TRN2
NeuronCores via Bass/Tile.

Sharding: the 64 (batch, head) pairs are split 8-per-core (pure data/head
parallelism, no cross-core communication). Each core runs the same program
(SPMD) on its own slice.

v2 design (vs the 405us baseline):
  - Q^T and K^T are pre-transposed to [d=128, S] ON THE HOST and shipped as
    bf16, killing all on-device QK transposes (256 PE matmuls + 256 DVE casts
    per core) and halving input DMA bytes.
  - All PE matmuls run in bf16 (1 cyc/row vs ~1.5 for f32r "HIGH" mode).
  - scores^T tiles [kv=128, q] land in [128, 1024] PSUM group tiles (2 kv
    blocks per group); ONE wide ScalarE exp per group halves the per-
    instruction ACTIVATE bubble count (20 vs 40 per pair).
  - softmax row sums use 4-way column-tiled matmuls: four ones-vector
    matmuls per 4 kv blocks write disjoint PSUM partitions {0,32,64,96}
    (tile_position auto-derived), executing concurrently on the PE's 32-col
    sub-arrays: ~4x less PE time than the baseline's 40 full-rate sums
    matmuls per pair.
  - causal masking: block-skip + narrowed sums/PV moving operands on the
    diagonal; the 128x128 diagonal triangle is masked multiplicatively on
    the OTHERWISE-IDLE GpSimd engine (P *= upper_tri) after exp.
  - finalize per chunk: sums rows are PE-transposed to [q, .] layout, one
    strided DVE reduce + reciprocal gives 1/rowsum, out^T is cast to bf16,
    PE-transposed, scaled per-partition by DVE, DMA'd out in [q, d] layout.
  - PV/sums emission lags scores/exp by 2 groups so the PE never drains
    (HAM stays at K=8/8: the baseline lost 36% of its time at half clock).
"""

import math
import sys

if "/opt/trn_rl_repo" not in sys.path:
    sys.path.insert(0, "/opt/trn_rl_repo")

import numpy as np
import ml_dtypes
from contextlib import ExitStack

import concourse.tile as tile
import concourse.mybir as mybir
from concourse import bacc
from concourse.bass_utils import run_bass_kernel_spmd
from concourse.masks import make_identity, make_upper_triangular

dt = mybir.dt
AF = mybir.ActivationFunctionType

B, H, S, D = 4, 16, 2048, 128
N_CORES = 8
PAIRS_PER_CORE = B * H // N_CORES
CHUNK = 512  # q columns per chunk (one PSUM bank of fp32)
BLK = 128  # kv block (partition dim)
GRP = 2  # kv blocks per exp group ([128, 1024] PSUM tile)
LAG = 3  # groups the PV/sums tail trails the scores/exp head

_cache = {}


def _build_attention_nc(n_pairs: int, seq: int) -> "bacc.Bacc":
    n_chunks = seq // CHUNK
    bpc = CHUNK // BLK  # kv blocks per chunk (4)
    scale = 1.0 / math.sqrt(D)

    nc = bacc.Bacc("TRN2", target_bir_lowering=False, debug=False)

    qt_d = nc.dram_tensor("qt", [n_pairs, D, seq], dt.bfloat16, kind="ExternalInput").ap()
    kt_d = nc.dram_tensor("kt", [n_pairs, D, seq], dt.bfloat16, kind="ExternalInput").ap()
    v_d = nc.dram_tensor("v", [n_pairs, seq, D], dt.bfloat16, kind="ExternalInput").ap()
    o_d = nc.dram_tensor("o", [n_pairs, seq, D], dt.float32, kind="ExternalOutput").ap()

    with tile.TileContext(nc) as tc, ExitStack() as ctx:
        const = ctx.enter_context(tc.tile_pool(name="const", bufs=1))
        stage = ctx.enter_context(tc.tile_pool(name="stage", bufs=2))
        ptp = ctx.enter_context(tc.tile_pool(name="ptp", bufs=5))
        sbf = ctx.enter_context(tc.tile_pool(name="sbf", bufs=2))
        # PSUM (8 banks): scores 2x[128,1024] (4 banks), out^T accumulator
        # 2x[128,512] (2 banks), sums 1 bank, finalize scratch 1 bank.
        ps_sc = ctx.enter_context(tc.tile_pool(name="ps_sc", bufs=2, space="PSUM"))
        ps_ot = ctx.enter_context(tc.tile_pool(name="ps_ot", bufs=2, space="PSUM"))
        ps_sum = ctx.enter_context(tc.tile_pool(name="ps_sum", bufs=1, space="PSUM"))
        ps_fin = ctx.enter_context(tc.tile_pool(name="ps_fin", bufs=1, space="PSUM"))

        ident = const.tile([128, 128], dt.float32)
        make_identity(nc, ident[:])
        ident_fr = const.tile([128, 128], dt.float32r)
        nc.vector.tensor_copy(ident_fr[:], ident[:])
        ident_bf = const.tile([128, 128], dt.bfloat16)
        nc.vector.tensor_copy(ident_bf[:], ident[:])
        ones_bf = const.tile([128, 1], dt.bfloat16)
        nc.vector.memset(ones_bf[:], 1.0)
        # multiplicative keep-mask for the diagonal 128x128 block in
        # [kv, q] layout: 1 where q >= kv (upper triangle incl diagonal)
        keep_f = const.tile([128, 128], dt.float32)
        make_upper_triangular(nc, keep_f[:], val=1.0, diag=True)
        keep_bf = const.tile([128, 128], dt.bfloat16)
        nc.vector.tensor_copy(keep_bf[:], keep_f[:])

        # PE + ScalarE warmup: the first pair's DMA takes ~10us with no PE
        # work, so HAM would hold the PE at K=4/8 (1.2 GHz) well into the
        # first pairs. Dummy matmuls (no DMA dependency) warm the clock
        # gate during the DMA; a dummy exp pulls the ACT table load
        # (~2.7us) off the critical path too.
        warm = ps_fin.tile([128, CHUNK], dt.float32, tag="fin")
        for w in range(160):
            nc.tensor.matmul(
                warm[:, 0:BLK], ident_bf[:], ident_bf[:], start=True, stop=True
            )
        warm_sb = sbf.tile([128, 4], dt.bfloat16, tag="warm_sb")
        nc.scalar.activation(warm_sb[:], warm[:, 0:4], AF.Exp, scale=0.001)

        # deferred tail entries: (pair, chunk, g, pt, tiles) emitted LAG
        # groups behind the scores/exp head so the PE never waits on exp.
        tail_q = []

        def emit_group_tail(ent):
            p, c, g, pt, pt_prev, otile, sums = ent
            jmax = bpc * (c + 1)
            qs = c * CHUNK
            # PV: out^T[d, q] += V_j^T P_j^T for this group's two blocks
            for b in range(GRP):
                j = GRP * g + b
                r = j - bpc * c  # >=0 only on the diagonal chunk
                off = r * BLK if j // bpc == c else 0
                nc.tensor.matmul(
                    otile[:, off:CHUNK],
                    vns[p % 2][:, j, :],
                    pt[:, b * CHUNK + off : (b + 1) * CHUNK],
                    start=(j == 0),
                    stop=(j == jmax - 1),
                )
            # sums: one 4-way column-tiled slot per pair of groups (4 kv
            # blocks); rows land on PSUM partitions {0,32,64,96} and run
            # concurrently on the PE's column sub-arrays.
            if g % 2 == 1:
                t = (g - 1) // 2
                if c == 0 and t == 0:
                    # diagonal-narrowed sums matmuls never write these; the
                    # finalize reduce reads full rows, so zero them here
                    # (inside the tail so the previous pair's deferred
                    # finalize has already read this bufs=1 bank).
                    for r in range(1, 4):
                        nc.vector.memset(sums[32 * r : 32 * r + 1, 0 : r * BLK], 0.0)
                for jj in range(4 * t, 4 * t + 4):
                    r = jj - bpc * c
                    off = r * BLK if jj // bpc == c else 0
                    src = pt_prev if jj < GRP * g else pt
                    b = jj % GRP
                    row = 32 * (jj % 4)
                    nc.tensor.matmul(
                        sums[row : row + 1, off:CHUNK],
                        ones_bf[:],
                        src[:, b * CHUNK + off : (b + 1) * CHUNK],
                        start=(t == 0),
                        stop=(t == c),
                        tile_position=(0, row),
                    )

        def emit_chunk_finalize(ent):
            p, c, otile, sums = ent
            qs = c * CHUNK
            # 1/rowsum: copy the sums bank to SBUF, PE-transpose to [q, .],
            # strided DVE reduce over the 4 partial rows, reciprocal.
            sums_sb = sbf.tile([128, CHUNK], dt.float32r, tag="sums_sb")
            nc.vector.tensor_copy(sums_sb[:], sums[:])
            fin = ps_fin.tile([128, CHUNK], dt.float32r, tag="fin")
            for b in range(bpc):
                nc.tensor.transpose(
                    fin[:, b * BLK : (b + 1) * BLK],
                    sums_sb[:, b * BLK : (b + 1) * BLK],
                    ident_fr[:],
                )
            rcp_in = sbf.tile([128, bpc], dt.float32, tag="rcp_in")
            nc.vector.tensor_reduce(
                rcp_in[:],
                fin.rearrange("p (b q) -> p b q", b=bpc)[:, :, 0:128:32],
                axis=mybir.AxisListType.X,
                op=mybir.AluOpType.add,
            )
            rcp = sbf.tile([128, bpc], dt.float32, tag="rcp")
            nc.vector.reciprocal(rcp[:], rcp_in[:])
            # out^T -> bf16 SBUF -> PE transpose -> per-partition scale -> DMA
            ot_sb = sbf.tile([128, CHUNK], dt.bfloat16, tag="ot_sb")
            nc.vector.tensor_copy(ot_sb[:], otile[:])
            fin2 = ps_fin.tile([128, CHUNK], dt.bfloat16, tag="fin")
            for b in range(bpc):
                nc.tensor.transpose(
                    fin2[:, b * BLK : (b + 1) * BLK],
                    ot_sb[:, b * BLK : (b + 1) * BLK],
                    ident_bf[:],
                )
            o_sb = sbf.tile([128, CHUNK], dt.float32, tag="o_sb")
            for b in range(bpc):
                nc.vector.tensor_scalar_mul(
                    o_sb[:, b * BLK : (b + 1) * BLK],
                    fin2[:, b * BLK : (b + 1) * BLK],
                    rcp[:, b : b + 1],
                )
            nc.sync.dma_start(
                out=o_d[p, qs : qs + CHUNK, :].rearrange("(b q) d -> q b d", q=BLK),
                in_=o_sb.rearrange("q (b d) -> q b d", b=bpc),
            )

        vns = [None, None]
        for p in range(n_pairs):
            qt = stage.tile([128, seq], dt.bfloat16, tag="qt")
            kt = stage.tile([128, seq], dt.bfloat16, tag="kt")
            vn = stage.tile([128, seq // BLK, D], dt.bfloat16, tag="vn")
            vns[p % 2] = vn
            nc.sync.dma_start(out=qt[:], in_=qt_d[p])
            nc.sync.dma_start(out=kt[:], in_=kt_d[p])
            nc.sync.dma_start(out=vn[:], in_=v_d[p].rearrange("(n p) d -> p n d", p=128))

            for c in range(n_chunks):
                qs = c * CHUNK
                n_grp = GRP * (c + 1)  # groups of 2 kv blocks each
                otile = ps_ot.tile([128, CHUNK], dt.float32, tag="ot")
                sums = ps_sum.tile([128, CHUNK], dt.float32, tag="sums")

                pt_prev = None
                for g in range(n_grp):
                    sc = ps_sc.tile([128, GRP * CHUNK], dt.float32, tag="sc")
                    diag = GRP * g + GRP - 1 >= bpc * c  # group touches diagonal
                    for b in range(GRP):
                        j = GRP * g + b
                        off = (j - bpc * c) * BLK if j >= bpc * c else 0
                        nc.tensor.matmul(
                            sc[:, b * CHUNK + off : (b + 1) * CHUNK],
                            kt[:, j * BLK : (j + 1) * BLK],
                            qt[:, qs + off : qs + CHUNK],
                            start=True,
                            stop=True,
                        )
                    pt = ptp.tile([128, GRP * CHUNK], dt.bfloat16, tag="pt")
                    if not diag:
                        nc.scalar.activation(pt[:], sc[:], AF.Exp, scale=scale)
                    else:
                        # per-block narrowed exp: read exactly the columns the
                        # (diagonal-narrowed) scores matmuls wrote
                        for b in range(GRP):
                            j = GRP * g + b
                            off = (j - bpc * c) * BLK if j >= bpc * c else 0
                            nc.scalar.activation(
                                pt[:, b * CHUNK + off : (b + 1) * CHUNK],
                                sc[:, b * CHUNK + off : (b + 1) * CHUNK],
                                AF.Exp,
                                scale=scale,
                            )
                    # mask the diagonal triangle (P *= upper_tri) on GpSimd
                    for b in range(GRP):
                        j = GRP * g + b
                        if j // bpc == c:
                            r = j - bpc * c
                            off = b * CHUNK + r * BLK
                            nc.gpsimd.tensor_mul(
                                pt[:, off : off + BLK],
                                pt[:, off : off + BLK],
                                keep_bf[:],
                            )
                    tail_q.append((p, c, g, pt, pt_prev, otile, sums))
                    pt_prev = pt
                    if len(tail_q) > LAG:
                        ent = tail_q.pop(0)
                        emit_group_tail(ent)
                        if ent[2] == GRP * (ent[1] + 1) - 1:  # last group of chunk
                            emit_chunk_finalize((ent[0], ent[1], ent[5], ent[6]))

        while tail_q:
            ent = tail_q.pop(0)
            emit_group_tail(ent)
            if ent[2] == GRP * (ent[1] + 1) - 1:
                emit_chunk_finalize((ent[0], ent[1], ent[5], ent[6]))

    nc.compile()
    return nc


def kernel(query_states, key_states, value_states, attention_mask):
    """Full-input entry point: shards (b,h) pairs across 8 NeuronCores,
    runs the Bass kernel SPMD, gathers the full output.

    attention_mask is the causal tril mask from the problem spec; causality
    is hardcoded in the device kernel, so the mask tensor is not shipped.
    Q and K are pre-transposed to [d, seq] and cast to bf16 on the host.
    """
    bf16 = ml_dtypes.bfloat16
    q = np.asarray(query_states, dtype=np.float32).reshape(B * H, S, D)
    k = np.asarray(key_states, dtype=np.float32).reshape(B * H, S, D)
    v = np.asarray(value_states, dtype=np.float32).reshape(B * H, S, D)
    qt = np.ascontiguousarray(q.transpose(0, 2, 1)).astype(bf16)
    kt = np.ascontiguousarray(k.transpose(0, 2, 1)).astype(bf16)
    vb = np.ascontiguousarray(v).astype(bf16)

    if "nc" not in _cache:
        _cache["nc"] = _build_attention_nc(PAIRS_PER_CORE, S)
    nc = _cache["nc"]

    in_maps = []
    for c in range(N_CORES):
        sl = slice(c * PAIRS_PER_CORE, (c + 1) * PAIRS_PER_CORE)
        in_maps.append(
            {
                "qt": np.ascontiguousarray(qt[sl]),
                "kt": np.ascontiguousarray(kt[sl]),
                "v": np.ascontiguousarray(vb[sl]),
            }
        )

    res = run_bass_kernel_spmd(nc, in_maps, list(range(N_CORES)))
    out = np.concatenate([res.results[c]["o"] for c in range(N_CORES)], axis=0)
    return out.reshape(B, H, S, D).astype(np.float32)


# revision 12
# speedup vs baseline: 1.0314x; 1.0314x over previous
"""Causal multi-head attention (B=4, H=16, S=2048, D=128, fp32) on 8 TRN2
NeuronCores via Bass/Tile.

Sharding: the 64 (batch, head) pairs are split 8-per-core (pure data/head
parallelism, no cross-core communication). Each core runs the same program
(SPMD) on its own slice.

v2 design (vs the 405us baseline):
  - Q^T and K^T are pre-transposed to [d=128, S] ON THE HOST and shipped as
    bf16, killing all on-device QK transposes (256 PE matmuls + 256 DVE casts
    per core) and halving input DMA bytes.
  - All PE matmuls run in bf16 (1 cyc/row vs ~1.5 for f32r "HIGH" mode).
  - scores^T tiles [kv=128, q] land in [128, 1024] PSUM group tiles (2 kv
    blocks per group); ONE wide ScalarE exp per group halves the per-
    instruction ACTIVATE bubble count (20 vs 40 per pair).
  - softmax row sums use 4-way column-tiled matmuls: four ones-vector
    matmuls per 4 kv blocks write disjoint PSUM partitions {0,32,64,96}
    (tile_position auto-derived), executing concurrently on the PE's 32-col
    sub-arrays: ~4x less PE time than the baseline's 40 full-rate sums
    matmuls per pair.
  - causal masking: block-skip + narrowed sums/PV moving operands on the
    diagonal; the 128x128 diagonal triangle is masked multiplicatively on
    the OTHERWISE-IDLE GpSimd engine (P *= upper_tri) after exp.
  - finalize per chunk: sums rows are PE-transposed to [q, .] layout, one
    strided DVE reduce + reciprocal gives 1/rowsum, out^T is cast to bf16,
    PE-transposed, scaled per-partition by DVE, DMA'd out in [q, d] layout.
  - PV/sums emission lags scores/exp by 2 groups so the PE never drains
    (HAM stays at K=8/8: the baseline lost 36% of its time at half clock).
"""

import math
import sys

if "/opt/trn_rl_repo" not in sys.path:
    sys.path.insert(0, "/opt/trn_rl_repo")

import numpy as np
import ml_dtypes
from contextlib import ExitStack

import concourse.tile as tile
import concourse.mybir as mybir
from concourse import bacc
from concourse.bass_utils import run_bass_kernel_spmd
from concourse.masks import make_identity, make_upper_triangular

dt = mybir.dt
AF = mybir.ActivationFunctionType

B, H, S, D = 4, 16, 2048, 128
N_CORES = 8
PAIRS_PER_CORE = B * H // N_CORES
CHUNK = 512  # q columns per chunk (one PSUM bank of fp32)
BLK = 128  # kv block (partition dim)
GRP = 2  # kv blocks per exp group ([128, 1024] PSUM tile)
LAG = 3  # groups the PV/sums tail trails the scores/exp head

_cache = {}


def _build_attention_nc(n_pairs: int, seq: int) -> "bacc.Bacc":
    n_chunks = seq // CHUNK
    bpc = CHUNK // BLK  # kv blocks per chunk (4)
    scale = 1.0 / math.sqrt(D)

    nc = bacc.Bacc("TRN2", target_bir_lowering=False, debug=False)

    qt_d = nc.dram_tensor("qt", [n_pairs, D, seq], dt.bfloat16, kind="ExternalInput").ap()
    kt_d = nc.dram_tensor("kt", [n_pairs, D, seq], dt.bfloat16, kind="ExternalInput").ap()
    v_d = nc.dram_tensor("v", [n_pairs, seq, D], dt.bfloat16, kind="ExternalInput").ap()
    o_d = nc.dram_tensor("o", [n_pairs, seq, D], dt.float32, kind="ExternalOutput").ap()

    with tile.TileContext(nc) as tc, ExitStack() as ctx:
        const = ctx.enter_context(tc.tile_pool(name="const", bufs=1))
        stage = ctx.enter_context(tc.tile_pool(name="stage", bufs=2))
        ptp = ctx.enter_context(tc.tile_pool(name="ptp", bufs=5))
        sbf = ctx.enter_context(tc.tile_pool(name="sbf", bufs=2))
        # PSUM (8 banks): scores 2x[128,1024] (4 banks), out^T accumulator
        # 2x[128,512] (2 banks), sums 1 bank, finalize scratch 1 bank.
        ps_sc = ctx.enter_context(tc.tile_pool(name="ps_sc", bufs=2, space="PSUM"))
        ps_ot = ctx.enter_context(tc.tile_pool(name="ps_ot", bufs=2, space="PSUM"))
        ps_sum = ctx.enter_context(tc.tile_pool(name="ps_sum", bufs=1, space="PSUM"))
        ps_fin = ctx.enter_context(tc.tile_pool(name="ps_fin", bufs=1, space="PSUM"))

        ident = const.tile([128, 128], dt.float32)
        make_identity(nc, ident[:])
        ident_fr = const.tile([128, 128], dt.float32r)
        nc.vector.tensor_copy(ident_fr[:], ident[:])
        ident_bf = const.tile([128, 128], dt.bfloat16)
        nc.vector.tensor_copy(ident_bf[:], ident[:])
        ones_bf = const.tile([128, 1], dt.bfloat16)
        nc.vector.memset(ones_bf[:], 1.0)
        # multiplicative keep-mask for the diagonal 128x128 block in
        # [kv, q] layout: 1 where q >= kv (upper triangle incl diagonal)
        keep_f = const.tile([128, 128], dt.float32)
        make_upper_triangular(nc, keep_f[:], val=1.0, diag=True)
        keep_bf = const.tile([128, 128], dt.bfloat16)
        nc.vector.tensor_copy(keep_bf[:], keep_f[:])

        # PE + ScalarE warmup: the first pair's DMA takes ~10us with no PE
        # work, so HAM would hold the PE at K=4/8 (1.2 GHz) well into the
        # first pairs. Dummy matmuls (no DMA dependency) warm the clock
        # gate during the DMA; a dummy exp pulls the ACT table load
        # (~2.7us) off the critical path too.
        warm = ps_fin.tile([128, CHUNK], dt.float32, tag="fin")
        for w in range(64):
            nc.tensor.matmul(
                warm[:, 0:BLK], ident_bf[:], ident_bf[:], start=True, stop=True
            )
        warm_sb = sbf.tile([128, 4], dt.bfloat16, tag="warm_sb")
        nc.scalar.activation(warm_sb[:], warm[:, 0:4], AF.Exp, scale=0.001)

        # deferred tail entries: (pair, chunk, g, pt, tiles) emitted LAG
        # groups behind the scores/exp head so the PE never waits on exp.
        tail_q = []

        def emit_group_tail(ent):
            p, c, g, pt, pt_prev, otile, sums = ent
            jmax = bpc * (c + 1)
            qs = c * CHUNK
            # PV: out^T[d, q] += V_j^T P_j^T for this group's two blocks
            for b in range(GRP):
                j = GRP * g + b
                r = j - bpc * c  # >=0 only on the diagonal chunk
                off = r * BLK if j // bpc == c else 0
                nc.tensor.matmul(
                    otile[:, off:CHUNK],
                    vns[p % 2][:, j, :],
                    pt[:, b * CHUNK + off : (b + 1) * CHUNK],
                    start=(j == 0),
                    stop=(j == jmax - 1),
                )
            # sums: one 4-way column-tiled slot per pair of groups (4 kv
            # blocks); rows land on PSUM partitions {0,32,64,96} and run
            # concurrently on the PE's column sub-arrays.
            if g % 2 == 1:
                t = (g - 1) // 2
                if c == 0 and t == 0:
                    # diagonal-narrowed sums matmuls never write these; the
                    # finalize reduce reads full rows, so zero them here
                    # (inside the tail so the previous pair's deferred
                    # finalize has already read this bufs=1 bank).
                    for r in range(1, 4):
                        nc.vector.memset(sums[32 * r : 32 * r + 1, 0 : r * BLK], 0.0)
                for jj in range(4 * t, 4 * t + 4):
                    r = jj - bpc * c
                    off = r * BLK if jj // bpc == c else 0
                    src = pt_prev if jj < GRP * g else pt
                    b = jj % GRP
                    row = 32 * (jj % 4)
                    nc.tensor.matmul(
                        sums[row : row + 1, off:CHUNK],
                        ones_bf[:],
                        src[:, b * CHUNK + off : (b + 1) * CHUNK],
                        start=(t == 0),
                        stop=(t == c),
                        tile_position=(0, row),
                    )

        def emit_chunk_finalize(ent):
            p, c, otile, sums = ent
            qs = c * CHUNK
            # 1/rowsum: copy the sums bank to SBUF, PE-transpose to [q, .],
            # strided DVE reduce over the 4 partial rows, reciprocal.
            sums_sb = sbf.tile([128, CHUNK], dt.float32r, tag="sums_sb")
            nc.vector.tensor_copy(sums_sb[:], sums[:])
            fin = ps_fin.tile([128, CHUNK], dt.float32r, tag="fin")
            for b in range(bpc):
                nc.tensor.transpose(
                    fin[:, b * BLK : (b + 1) * BLK],
                    sums_sb[:, b * BLK : (b + 1) * BLK],
                    ident_fr[:],
                )
            rcp_in = sbf.tile([128, bpc], dt.float32, tag="rcp_in")
            nc.vector.tensor_reduce(
                rcp_in[:],
                fin.rearrange("p (b q) -> p b q", b=bpc)[:, :, 0:128:32],
                axis=mybir.AxisListType.X,
                op=mybir.AluOpType.add,
            )
            rcp = sbf.tile([128, bpc], dt.float32, tag="rcp")
            nc.vector.reciprocal(rcp[:], rcp_in[:])
            # out^T -> bf16 SBUF -> PE transpose -> per-partition scale -> DMA
            ot_sb = sbf.tile([128, CHUNK], dt.bfloat16, tag="ot_sb")
            nc.vector.tensor_copy(ot_sb[:], otile[:])
            fin2 = ps_fin.tile([128, CHUNK], dt.bfloat16, tag="fin")
            for b in range(bpc):
                nc.tensor.transpose(
                    fin2[:, b * BLK : (b + 1) * BLK],
                    ot_sb[:, b * BLK : (b + 1) * BLK],
                    ident_bf[:],
                )
            o_sb = sbf.tile([128, CHUNK], dt.float32, tag="o_sb")
            for b in range(bpc):
                nc.vector.tensor_scalar_mul(
                    o_sb[:, b * BLK : (b + 1) * BLK],
                    fin2[:, b * BLK : (b + 1) * BLK],
                    rcp[:, b : b + 1],
                )
            nc.sync.dma_start(
                out=o_d[p, qs : qs + CHUNK, :].rearrange("(b q) d -> q b d", q=BLK),
                in_=o_sb.rearrange("q (b d) -> q b d", b=bpc),
            )

        vns = [None, None]
        for p in range(n_pairs):
            qt = stage.tile([128, seq], dt.bfloat16, tag="qt")
            kt = stage.tile([128, seq], dt.bfloat16, tag="kt")
            vn = stage.tile([128, seq // BLK, D], dt.bfloat16, tag="vn")
            vns[p % 2] = vn
            # halved DMAs: the first chunks only need the first half, so
            # compute starts as soon as half the data has landed
            hs = seq // 2
            hn = hs // BLK
            nc.sync.dma_start(out=kt[:, 0:hs], in_=kt_d[p, :, 0:hs])
            nc.sync.dma_start(out=qt[:, 0:hs], in_=qt_d[p, :, 0:hs])
            nc.sync.dma_start(
                out=vn[:, 0:hn, :],
                in_=v_d[p, 0:hs, :].rearrange("(n p) d -> p n d", p=128),
            )
            nc.sync.dma_start(out=kt[:, hs:seq], in_=kt_d[p, :, hs:seq])
            nc.sync.dma_start(out=qt[:, hs:seq], in_=qt_d[p, :, hs:seq])
            nc.sync.dma_start(
                out=vn[:, hn:, :],
                in_=v_d[p, hs:seq, :].rearrange("(n p) d -> p n d", p=128),
            )

            for c in range(n_chunks):
                qs = c * CHUNK
                n_grp = GRP * (c + 1)  # groups of 2 kv blocks each
                otile = ps_ot.tile([128, CHUNK], dt.float32, tag="ot")
                sums = ps_sum.tile([128, CHUNK], dt.float32, tag="sums")

                pt_prev = None
                for g in range(n_grp):
                    sc = ps_sc.tile([128, GRP * CHUNK], dt.float32, tag="sc")
                    diag = GRP * g + GRP - 1 >= bpc * c  # group touches diagonal
                    for b in range(GRP):
                        j = GRP * g + b
                        off = (j - bpc * c) * BLK if j >= bpc * c else 0
                        nc.tensor.matmul(
                            sc[:, b * CHUNK + off : (b + 1) * CHUNK],
                            kt[:, j * BLK : (j + 1) * BLK],
                            qt[:, qs + off : qs + CHUNK],
                            start=True,
                            stop=True,
                        )
                    pt = ptp.tile([128, GRP * CHUNK], dt.bfloat16, tag="pt")
                    if not diag:
                        nc.scalar.activation(pt[:], sc[:], AF.Exp, scale=scale)
                    else:
                        # per-block narrowed exp: read exactly the columns the
                        # (diagonal-narrowed) scores matmuls wrote
                        for b in range(GRP):
                            j = GRP * g + b
                            off = (j - bpc * c) * BLK if j >= bpc * c else 0
                            nc.scalar.activation(
                                pt[:, b * CHUNK + off : (b + 1) * CHUNK],
                                sc[:, b * CHUNK + off : (b + 1) * CHUNK],
                                AF.Exp,
                                scale=scale,
                            )
                    # mask the diagonal triangle (P *= upper_tri) on GpSimd
                    for b in range(GRP):
                        j = GRP * g + b
                        if j // bpc == c:
                            r = j - bpc * c
                            off = b * CHUNK + r * BLK
                            nc.gpsimd.tensor_mul(
                                pt[:, off : off + BLK],
                                pt[:, off : off + BLK],
                                keep_bf[:],
                            )
                    tail_q.append((p, c, g, pt, pt_prev, otile, sums))
                    pt_prev = pt
                    if len(tail_q) > LAG:
                        ent = tail_q.pop(0)
                        emit_group_tail(ent)
                        if ent[2] == GRP * (ent[1] + 1) - 1:  # last group of chunk
                            emit_chunk_finalize((ent[0], ent[1], ent[5], ent[6]))

        while tail_q:
            ent = tail_q.pop(0)
            emit_group_tail(ent)
            if ent[2] == GRP * (ent[1] + 1) - 1:
                emit_chunk_finalize((ent[0], ent[1], ent[5], ent[6]))

    nc.compile()
    return nc


def kernel(query_states, key_states, value_states, attention_mask):
    """Full-input entry point: shards (b,h) pairs across 8 NeuronCores,
    runs the Bass kernel SPMD, gathers the full output.

    attention_mask is the causal tril mask from the problem spec; causality
    is hardcoded in the device kernel, so the mask tensor is not shipped.
    Q and K are pre-transposed to [d, seq] and cast to bf16 on the host.
    """
    bf16 = ml_dtypes.bfloat16
    q = np.asarray(query_states, dtype=np.float32).reshape(B * H, S, D)
    k = np.asarray(key_states, dtype=np.float32).reshape(B * H, S, D)
    v = np.asarray(value_states, dtype=np.float32).reshape(B * H, S, D)
    qt = np.ascontiguousarray(q.transpose(0, 2, 1)).astype(bf16)
    kt = np.ascontiguousarray(k.transpose(0, 2, 1)).astype(bf16)
    vb = np.ascontiguousarray(v).astype(bf16)

    if "nc" not in _cache:
        _cache["nc"] = _build_attention_nc(PAIRS_PER_CORE, S)
    nc = _cache["nc"]

    in_maps = []
    for c in range(N_CORES):
        sl = slice(c * PAIRS_PER_CORE, (c + 1) * PAIRS_PER_CORE)
        in_maps.append(
            {
                "qt": np.ascontiguousarray(qt[sl]),
                "kt": np.ascontiguousarray(kt[sl]),
                "v": np.ascontiguousarray(vb[sl]),
            }
        )

    res = run_bass_kernel_spmd(nc, in_maps, list(range(N_CORES)))
    out = np.concatenate([res.results[c]["o"] for c in range(N_CORES)], axis=0)
    return out.reshape(B, H, S, D).astype(np.float32)


# revision 14
# speedup vs baseline: 1.0435x; 1.0116x over previous
"""Causal multi-head attention (B=4, H=16, S=2048, D=128, fp32) on 8 TRN2
NeuronCores via Bass/Tile.

Sharding: the 64 (batch, head) pairs are split 8-per-core (pure data/head
parallelism, no cross-core communication). Each core runs the same program
(SPMD) on its own slice.

v2 design (vs the 405us baseline):
  - Q^T and K^T are pre-transposed to [d=128, S] ON THE HOST and shipped as
    bf16, killing all on-device QK transposes (256 PE matmuls + 256 DVE casts
    per core) and halving input DMA bytes.
  - All PE matmuls run in bf16 (1 cyc/row vs ~1.5 for f32r "HIGH" mode).
  - scores^T tiles [kv=128, q] land in [128, 1024] PSUM group tiles (2 kv
    blocks per group); ONE wide ScalarE exp per group halves the per-
    instruction ACTIVATE bubble count (20 vs 40 per pair).
  - softmax row sums use 4-way column-tiled matmuls: four ones-vector
    matmuls per 4 kv blocks write disjoint PSUM partitions {0,32,64,96}
    (tile_position auto-derived), executing concurrently on the PE's 32-col
    sub-arrays: ~4x less PE time than the baseline's 40 full-rate sums
    matmuls per pair.
  - causal masking: block-skip + narrowed sums/PV moving operands on the
    diagonal; the 128x128 diagonal triangle is masked multiplicatively on
    the OTHERWISE-IDLE GpSimd engine (P *= upper_tri) after exp.
  - finalize per chunk: sums rows are PE-transposed to [q, .] layout, one
    strided DVE reduce + reciprocal gives 1/rowsum, out^T is cast to bf16,
    PE-transposed, scaled per-partition by DVE, DMA'd out in [q, d] layout.
  - PV/sums emission lags scores/exp by 2 groups so the PE never drains
    (HAM stays at K=8/8: the baseline lost 36% of its time at half clock).
"""

import math
import sys

if "/opt/trn_rl_repo" not in sys.path:
    sys.path.insert(0, "/opt/trn_rl_repo")

import numpy as np
import ml_dtypes
from contextlib import ExitStack

import concourse.tile as tile
import concourse.mybir as mybir
from concourse import bacc
from concourse.bass_utils import run_bass_kernel_spmd
from concourse.masks import make_identity, make_upper_triangular

dt = mybir.dt
AF = mybir.ActivationFunctionType

B, H, S, D = 4, 16, 2048, 128
N_CORES = 8
PAIRS_PER_CORE = B * H // N_CORES
CHUNK = 512  # q columns per chunk (one PSUM bank of fp32)
BLK = 128  # kv block (partition dim)
GRP = 2  # kv blocks per exp group ([128, 1024] PSUM tile)
LAG = 3  # groups the PV/sums tail trails the scores/exp head

_cache = {}


def _build_attention_nc(n_pairs: int, seq: int) -> "bacc.Bacc":
    n_chunks = seq // CHUNK
    bpc = CHUNK // BLK  # kv blocks per chunk (4)
    scale = 1.0 / math.sqrt(D)

    nc = bacc.Bacc("TRN2", target_bir_lowering=False, debug=False)

    qt_d = nc.dram_tensor("qt", [n_pairs, D, seq], dt.bfloat16, kind="ExternalInput").ap()
    kt_d = nc.dram_tensor("kt", [n_pairs, D, seq], dt.bfloat16, kind="ExternalInput").ap()
    v_d = nc.dram_tensor("v", [n_pairs, seq, D], dt.bfloat16, kind="ExternalInput").ap()
    o_d = nc.dram_tensor("o", [n_pairs, seq, D], dt.float32, kind="ExternalOutput").ap()

    with tile.TileContext(nc) as tc, ExitStack() as ctx:
        const = ctx.enter_context(tc.tile_pool(name="const", bufs=1))
        stage = ctx.enter_context(tc.tile_pool(name="stage", bufs=2))
        ptp = ctx.enter_context(tc.tile_pool(name="ptp", bufs=5))
        sbf = ctx.enter_context(tc.tile_pool(name="sbf", bufs=2))
        # PSUM (8 banks): scores 2x[128,1024] (4 banks), out^T accumulator
        # 2x[128,512] (2 banks), sums 1 bank, finalize scratch 1 bank.
        ps_sc = ctx.enter_context(tc.tile_pool(name="ps_sc", bufs=2, space="PSUM"))
        ps_ot = ctx.enter_context(tc.tile_pool(name="ps_ot", bufs=2, space="PSUM"))
        ps_sum = ctx.enter_context(tc.tile_pool(name="ps_sum", bufs=1, space="PSUM"))
        ps_fin = ctx.enter_context(tc.tile_pool(name="ps_fin", bufs=1, space="PSUM"))

        ident = const.tile([128, 128], dt.float32)
        make_identity(nc, ident[:])
        ident_fr = const.tile([128, 128], dt.float32r)
        nc.vector.tensor_copy(ident_fr[:], ident[:])
        ident_bf = const.tile([128, 128], dt.bfloat16)
        nc.vector.tensor_copy(ident_bf[:], ident[:])
        ones_bf = const.tile([128, 1], dt.bfloat16)
        nc.vector.memset(ones_bf[:], 1.0)
        # multiplicative keep-mask for the diagonal 128x128 block in
        # [kv, q] layout: 1 where q >= kv (upper triangle incl diagonal)
        keep_f = const.tile([128, 128], dt.float32)
        make_upper_triangular(nc, keep_f[:], val=1.0, diag=True)
        keep_bf = const.tile([128, 128], dt.bfloat16)
        nc.vector.tensor_copy(keep_bf[:], keep_f[:])

        # ScalarE warmup: a dummy exp on a const pulls the ACT table load
        # (~2.7us) off the critical path, overlapping the first DMA.
        warm_sb = sbf.tile([128, 4], dt.bfloat16, tag="warm_sb")
        nc.scalar.activation(warm_sb[:], ident[:, 0:4], AF.Exp, scale=0.001)

        # deferred tail entries: (pair, chunk, g, pt, tiles) emitted LAG
        # groups behind the scores/exp head so the PE never waits on exp.
        tail_q = []

        def emit_group_tail(ent):
            p, c, g, pt, pt_prev, otile, sums = ent
            jmax = bpc * (c + 1)
            qs = c * CHUNK
            # PV: out^T[d, q] += V_j^T P_j^T for this group's two blocks
            for b in range(GRP):
                j = GRP * g + b
                r = j - bpc * c  # >=0 only on the diagonal chunk
                off = r * BLK if j // bpc == c else 0
                nc.tensor.matmul(
                    otile[:, off:CHUNK],
                    vns[p % 2][:, j, :],
                    pt[:, b * CHUNK + off : (b + 1) * CHUNK],
                    start=(j == 0),
                    stop=(j == jmax - 1),
                )
            # sums: one 4-way column-tiled slot per pair of groups (4 kv
            # blocks); rows land on PSUM partitions {0,32,64,96} and run
            # concurrently on the PE's column sub-arrays.
            if g % 2 == 1:
                t = (g - 1) // 2
                if c == 0 and t == 0:
                    # diagonal-narrowed sums matmuls never write these; the
                    # finalize reduce reads full rows, so zero them here
                    # (inside the tail so the previous pair's deferred
                    # finalize has already read this bufs=1 bank).
                    for r in range(1, 4):
                        nc.vector.memset(sums[32 * r : 32 * r + 1, 0 : r * BLK], 0.0)
                for jj in range(4 * t, 4 * t + 4):
                    r = jj - bpc * c
                    off = r * BLK if jj // bpc == c else 0
                    src = pt_prev if jj < GRP * g else pt
                    b = jj % GRP
                    row = 32 * (jj % 4)
                    nc.tensor.matmul(
                        sums[row : row + 1, off:CHUNK],
                        ones_bf[:],
                        src[:, b * CHUNK + off : (b + 1) * CHUNK],
                        start=(t == 0),
                        stop=(t == c),
                        tile_position=(0, row),
                    )

        def emit_chunk_finalize(ent):
            p, c, otile, sums = ent
            qs = c * CHUNK
            # 1/rowsum: copy the sums bank to SBUF, PE-transpose to [q, .],
            # strided DVE reduce over the 4 partial rows, reciprocal.
            sums_sb = sbf.tile([128, CHUNK], dt.float32r, tag="sums_sb")
            nc.vector.tensor_copy(sums_sb[:], sums[:])
            fin = ps_fin.tile([128, CHUNK], dt.float32r, tag="fin")
            for b in range(bpc):
                nc.tensor.transpose(
                    fin[:, b * BLK : (b + 1) * BLK],
                    sums_sb[:, b * BLK : (b + 1) * BLK],
                    ident_fr[:],
                )
            rcp_in = sbf.tile([128, bpc], dt.float32, tag="rcp_in")
            nc.vector.tensor_reduce(
                rcp_in[:],
                fin.rearrange("p (b q) -> p b q", b=bpc)[:, :, 0:128:32],
                axis=mybir.AxisListType.X,
                op=mybir.AluOpType.add,
            )
            rcp = sbf.tile([128, bpc], dt.float32, tag="rcp")
            nc.vector.reciprocal(rcp[:], rcp_in[:])
            # out^T -> bf16 SBUF -> PE transpose -> per-partition scale -> DMA
            ot_sb = sbf.tile([128, CHUNK], dt.bfloat16, tag="ot_sb")
            nc.vector.tensor_copy(ot_sb[:], otile[:])
            fin2 = ps_fin.tile([128, CHUNK], dt.bfloat16, tag="fin")
            for b in range(bpc):
                nc.tensor.transpose(
                    fin2[:, b * BLK : (b + 1) * BLK],
                    ot_sb[:, b * BLK : (b + 1) * BLK],
                    ident_bf[:],
                )
            o_sb = sbf.tile([128, CHUNK], dt.float32, tag="o_sb")
            for b in range(bpc):
                nc.vector.tensor_scalar_mul(
                    o_sb[:, b * BLK : (b + 1) * BLK],
                    fin2[:, b * BLK : (b + 1) * BLK],
                    rcp[:, b : b + 1],
                )
            nc.sync.dma_start(
                out=o_d[p, qs : qs + CHUNK, :].rearrange("(b q) d -> q b d", q=BLK),
                in_=o_sb.rearrange("q (b d) -> q b d", b=bpc),
            )

        vns = [None, None]
        for p in range(n_pairs):
            qt = stage.tile([128, seq], dt.bfloat16, tag="qt")
            kt = stage.tile([128, seq], dt.bfloat16, tag="kt")
            vn = stage.tile([128, seq // BLK, D], dt.bfloat16, tag="vn")
            vns[p % 2] = vn
            # halved DMAs: the first chunks only need the first half, so
            # compute starts as soon as half the data has landed
            hs = seq // 2
            hn = hs // BLK
            nc.sync.dma_start(out=kt[:, 0:hs], in_=kt_d[p, :, 0:hs])
            nc.sync.dma_start(out=qt[:, 0:hs], in_=qt_d[p, :, 0:hs])
            nc.sync.dma_start(
                out=vn[:, 0:hn, :],
                in_=v_d[p, 0:hs, :].rearrange("(n p) d -> p n d", p=128),
            )
            nc.sync.dma_start(out=kt[:, hs:seq], in_=kt_d[p, :, hs:seq])
            nc.sync.dma_start(out=qt[:, hs:seq], in_=qt_d[p, :, hs:seq])
            nc.sync.dma_start(
                out=vn[:, hn:, :],
                in_=v_d[p, hs:seq, :].rearrange("(n p) d -> p n d", p=128),
            )

            for c in range(n_chunks):
                qs = c * CHUNK
                n_grp = GRP * (c + 1)  # groups of 2 kv blocks each
                otile = ps_ot.tile([128, CHUNK], dt.float32, tag="ot")
                sums = ps_sum.tile([128, CHUNK], dt.float32, tag="sums")

                pt_prev = None
                for g in range(n_grp):
                    sc = ps_sc.tile([128, GRP * CHUNK], dt.float32, tag="sc")
                    diag = GRP * g + GRP - 1 >= bpc * c  # group touches diagonal
                    for b in range(GRP):
                        j = GRP * g + b
                        off = (j - bpc * c) * BLK if j >= bpc * c else 0
                        nc.tensor.matmul(
                            sc[:, b * CHUNK + off : (b + 1) * CHUNK],
                            kt[:, j * BLK : (j + 1) * BLK],
                            qt[:, qs + off : qs + CHUNK],
                            start=True,
                            stop=True,
                        )
                    pt = ptp.tile([128, GRP * CHUNK], dt.bfloat16, tag="pt")
                    first_off = (GRP * g - bpc * c) * BLK if GRP * g >= bpc * c else 0
                    if not diag or first_off < GRP * BLK:
                        # one wide exp; for the r0/r1 diagonal group the
                        # per-inst ACT bubble outweighs the <=128 stale
                        # columns it reads (their pt output is never
                        # consumed: PV/sums moving operands are narrowed)
                        nc.scalar.activation(
                            pt[:, first_off:], sc[:, first_off:], AF.Exp, scale=scale
                        )
                    else:
                        # r2/r3 diagonal group: narrow per-block exps win
                        for b in range(GRP):
                            j = GRP * g + b
                            off = (j - bpc * c) * BLK
                            nc.scalar.activation(
                                pt[:, b * CHUNK + off : (b + 1) * CHUNK],
                                sc[:, b * CHUNK + off : (b + 1) * CHUNK],
                                AF.Exp,
                                scale=scale,
                            )
                    # mask the diagonal triangle (P *= upper_tri) on GpSimd
                    for b in range(GRP):
                        j = GRP * g + b
                        if j // bpc == c:
                            r = j - bpc * c
                            off = b * CHUNK + r * BLK
                            nc.gpsimd.tensor_mul(
                                pt[:, off : off + BLK],
                                pt[:, off : off + BLK],
                                keep_bf[:],
                            )
                    tail_q.append((p, c, g, pt, pt_prev, otile, sums))
                    pt_prev = pt
                    if len(tail_q) > LAG:
                        ent = tail_q.pop(0)
                        emit_group_tail(ent)
                        if ent[2] == GRP * (ent[1] + 1) - 1:  # last group of chunk
                            emit_chunk_finalize((ent[0], ent[1], ent[5], ent[6]))

        while tail_q:
            ent = tail_q.pop(0)
            emit_group_tail(ent)
            if ent[2] == GRP * (ent[1] + 1) - 1:
                emit_chunk_finalize((ent[0], ent[1], ent[5], ent[6]))

    nc.compile()
    return nc


def kernel(query_states, key_states, value_states, attention_mask):
    """Full-input entry point: shards (b,h) pairs across 8 NeuronCores,
    runs the Bass kernel SPMD, gathers the full output.

    attention_mask is the causal tril mask from the problem spec; causality
    is hardcoded in the device kernel, so the mask tensor is not shipped.
    Q and K are pre-transposed to [d, seq] and cast to bf16 on the host.
    """
    bf16 = ml_dtypes.bfloat16
    q = np.asarray(query_states, dtype=np.float32).reshape(B * H, S, D)
    k = np.asarray(key_states, dtype=np.float32).reshape(B * H, S, D)
    v = np.asarray(value_states, dtype=np.float32).reshape(B * H, S, D)
    qt = np.ascontiguousarray(q.transpose(0, 2, 1)).astype(bf16)
    kt = np.ascontiguousarray(k.transpose(0, 2, 1)).astype(bf16)
    vb = np.ascontiguousarray(v).astype(bf16)

    if "nc" not in _cache:
        _cache["nc"] = _build_attention_nc(PAIRS_PER_CORE, S)
    nc = _cache["nc"]

    in_maps = []
    for c in range(N_CORES):
        sl = slice(c * PAIRS_PER_CORE, (c + 1) * PAIRS_PER_CORE)
        in_maps.append(
            {
                "qt": np.ascontiguousarray(qt[sl]),
                "kt": np.ascontiguousarray(kt[sl]),
                "v": np.ascontiguousarray(vb[sl]),
            }
        )

    res = run_bass_kernel_spmd(nc, in_maps, list(range(N_CORES)))
    out = np.concatenate([res.results[c]["o"] for c in range(N_CORES)], axis=0)
    return out.reshape(B, H, S, D).astype(np.float32)
